# revision 11
# baseline (speedup 1.0000x reference)
"""Trainium2 Bass kernel for the 21-joint hand-graph message-passing MLP.

Math (per sample b, per target joint t with neighbor list S_t of length n):
    g   = concat(x[b, S_t[0]], ..., x[b, S_t[n-1]])          # [n*64]
    h1  = relu(g @ W1_t + b1_t)                              # [128]
    h2  = relu(h1 @ W2_t + b2_t)                             # [128]
    out[b, t] = h2 @ W3_t + b3_t                             # [64]

Strategy (pure data parallel over 8 NeuronCores, B=65536 -> 8192/core):
  - The host pre-packs x to bf16 in a 22-slot node order SEQ so that the
    11 adjacent slot pairs are exactly the pair tiles the L1 chunk plan
    wants: (0,1),(1,2),(3,4),(5,6),(7,8),(9,10),(11,12),(13,14),(15,16),
    (17,18),(19,20).  Per batch tile, ONE xbar DMA-transpose of
    [TILE, 22*64] -> [128, 11, TILE] (3D out: dim 1 = source column
    group) produces all 11 feature-major pair tiles in a single
    instruction, keeping the Sync sequencer off the critical path.
    Per-core HBM traffic: 22.6MB read + 21.5MB write.
  - L1 runs weight-stationary with K=128 chunks: a chunk is either a
    full pair (both nodes of a tile in the neighbor list) or a single
    (one node, other 64 weight rows zero).  47 chunks total.
  - L1/L2 relu+bias are fused into the PSUM->SBUF evacuation, alternated
    between ScalarE and VectorE (the only PSUM readers).
  - L3 is W3-stationary with the output FEATURE-major: two targets share
    one PSUM bank via column tiling (psum[0:64]=target a, psum[64:128]=b),
    N=512 per matmul.  b3 is added during evacuation (per-partition
    bias).  The store is a plain bf16 DMA to out[1344, BC] on the GpSimd
    SWDGE path; the host does the final [B,21,64] transpose + fp32 cast.
"""

import os
import numpy as np
import ml_dtypes

B, J, D, H1, H2 = 65536, 21, 64, 128, 128
NCORES = 8
BC = B // NCORES          # 8192 samples per core
TILE = 1024               # batch tile (2 PSUM banks wide in fp32)
NTILES = BC // TILE       # 8

FINGER_BASE = [4 * f + 1 for f in range(5)]
NEIGH = {
    6: [[0, 1, 5, 9, 13, 17]],
    5: [[0, 5, 6, 1, 9], [0, 9, 10, 5, 13], [0, 13, 14, 9, 17]],
    4: [[0, 1, 2, 5], [0, 17, 18, 13]],
    3: [r for b in FINGER_BASE for r in ([b, b + 1, b + 2], [b + 1, b + 2, b + 3])],
    2: [[b + 2, b + 3] for b in FINGER_BASE],
}
OUT = {
    6: [0],
    5: [5, 9, 13],
    4: [1, 17],
    3: [j for b in FINGER_BASE for j in (b + 1, b + 2)],
    2: [b + 3 for b in FINGER_BASE],
}
GROUPS = [6, 5, 4, 3, 2]

# target t -> (n, row index within its group, neighbor list)
TARGET = {}
for n in GROUPS:
    for row, t in enumerate(OUT[n]):
        TARGET[t] = (n, row, list(NEIGH[n][row]))

# Pair tiles (ARBITRARY node pairs - the host packs x in this slot order)
# chosen so the per-target max matching reaches the 41-chunk floor.
PAIRS = [(0, 1), (1, 2), (2, 5), (3, 4), (5, 6), (7, 8), (9, 10),
         (11, 12), (13, 14), (15, 16), (17, 18), (19, 20),
         (5, 9), (13, 17), (0, 13), (9, 17)]
NPAIRS = len(PAIRS)
SLOTS = 2 * NPAIRS                      # 32 node slots in xbf
SEQ = [j for p in PAIRS for j in p]     # slot s holds node SEQ[s]
PAIR_IDX = {p: i for i, p in enumerate(PAIRS)}

# node -> (tile, half) placement (first occurrence wins for singles)
NODE_SLOTS = {}
for i, (a, b) in enumerate(PAIRS):
    NODE_SLOTS.setdefault(a, []).append((i, 0))
    NODE_SLOTS.setdefault(b, []).append((i, 1))


def build_chunk_plan():
    """Per target, split neighbor positions into K=128 contraction chunks.

    chunk = dict(tile, slots) with slots = (pos_or_None for half 0,
    pos_or_None for half 1); position i covers W1 rows 64*i : 64*i+64.
    """
    plan = {}
    for t in range(21):
        n, _, S = TARGET[t]

        # exhaustive max matching of disjoint in-set pairs (n <= 6)
        best = []

        def rec(pos, used, acc):
            nonlocal best
            if len(acc) > len(best):
                best = list(acc)
            for i in range(pos, n):
                if used[i]:
                    continue
                for k in range(i + 1, n):
                    if used[k]:
                        continue
                    a, b = S[i], S[k]
                    lo, hi = min(a, b), max(a, b)
                    if (lo, hi) in PAIR_IDX:
                        pi, pk = (i, k) if a == lo else (k, i)
                        used[i] = used[k] = True
                        acc.append((PAIR_IDX[(lo, hi)], pi, pk))
                        rec(i + 1, used, acc)
                        acc.pop()
                        used[i] = used[k] = False
            return

        rec(0, [False] * n, [])
        chunks = []
        used = [False] * n
        for tile_idx, pi, pk in best:
            chunks.append(dict(tile=tile_idx, slots=(pi, pk)))
            used[pi] = used[pk] = True
        for i in range(n):
            if not used[i]:
                tile_idx, half = NODE_SLOTS[S[i]][0]
                slots = (i, None) if half == 0 else (None, i)
                chunks.append(dict(tile=tile_idx, slots=slots))
        plan[t] = chunks
    return plan


CHUNK_PLAN = build_chunk_plan()
TOTAL_CHUNKS = sum(len(v) for v in CHUNK_PLAN.values())     # 47

# deterministic column layout of packed W1 chunks: order by (t, ci)
CHUNK_COLS = {}
_col = 0
for _t in range(21):
    for _ci in range(len(CHUNK_PLAN[_t])):
        CHUNK_COLS[(_t, _ci)] = _col
        _col += 128

# L3 pairs of targets sharing one PSUM bank via column tiling
L3_PAIRS = [(2 * i, 2 * i + 1) for i in range(10)] + [(20,)]


def pack_weights(inputs):
    """Host-side prep: permute/pack all weights into a handful of flat arrays."""
    bf16 = ml_dtypes.bfloat16
    w1p = np.zeros((128, 128 * TOTAL_CHUNKS), np.float32)
    for t in range(21):
        n, row, S = TARGET[t]
        W1 = np.asarray(inputs[f"w1_g{n}"][row], np.float32)  # [n*64, 128]
        for ci, ch in enumerate(CHUNK_PLAN[t]):
            col = CHUNK_COLS[(t, ci)]
            for half, pos in enumerate(ch["slots"]):
                if pos is not None:
                    w1p[64 * half:64 * half + 64, col:col + 128] = \
                        W1[64 * pos:64 * pos + 64]
    w2p = np.zeros((128, 128 * 21), np.float32)
    w3p = np.zeros((128, 64 * 21), np.float32)
    b1p = np.zeros((128, 21), np.float32)
    b2p = np.zeros((128, 21), np.float32)
    b3p = np.zeros((128, len(L3_PAIRS)), np.float32)
    for t in range(21):
        n, row, _ = TARGET[t]
        w2p[:, 128 * t:128 * (t + 1)] = np.asarray(inputs[f"w2_g{n}"][row])
        w3p[:, 64 * t:64 * (t + 1)] = np.asarray(inputs[f"w3_g{n}"][row])
        b1p[:, t] = np.asarray(inputs[f"b1_g{n}"][row])
        b2p[:, t] = np.asarray(inputs[f"b2_g{n}"][row])
    for pi, pr in enumerate(L3_PAIRS):
        for k, t in enumerate(pr):
            n, row, _ = TARGET[t]
            b3p[64 * k:64 * k + 64, pi] = np.asarray(inputs[f"b3_g{n}"][row])
    return dict(
        w1p=w1p.astype(bf16), w2p=w2p.astype(bf16), w3p=w3p.astype(bf16),
        b1p=b1p, b2p=b2p, b3p=b3p,
    )


def numpy_emulate(inputs, x):
    """Bit-layout-faithful numpy model of what the HW kernel computes (minus
    PSUM rounding): used to validate the plan / packing offline."""
    bf16 = ml_dtypes.bfloat16
    packed = pack_weights(inputs)
    xb = np.asarray(x, np.float32).astype(bf16)          # [Bn, 21, 64]
    Bn = x.shape[0]
    xT = {}
    for i, (a, b) in enumerate(PAIRS):
        xT[i] = np.concatenate([xb[:, a], xb[:, b]], 1).T  # [128, Bn]
    out = np.zeros((Bn, 21, 64), np.float32)
    for t in range(21):
        psum1 = np.zeros((128, Bn), np.float32)
        for ci, ch in enumerate(CHUNK_PLAN[t]):
            col = CHUNK_COLS[(t, ci)]
            lhsT = packed["w1p"][:, col:col + 128].astype(np.float32)
            rhs = xT[ch["tile"]].astype(np.float32)
            psum1 += lhsT.T @ rhs
        h1 = np.maximum(psum1 + packed["b1p"][:, t:t + 1], 0).astype(bf16)
        w2 = packed["w2p"][:, 128 * t:128 * (t + 1)].astype(np.float32)
        psum2 = w2.T @ h1.astype(np.float32)
        h2 = np.maximum(psum2 + packed["b2p"][:, t:t + 1], 0).astype(bf16)
        w3 = packed["w3p"][:, 64 * t:64 * (t + 1)].astype(np.float32)
        psum3 = w3.T @ h2.astype(np.float32)             # [64, Bn]
        pi, k = (t // 2, t % 2) if t < 20 else (10, 0)
        b3 = packed["b3p"][64 * k:64 * k + 64, pi]
        out[:, t] = (psum3 + b3[:, None]).astype(bf16).astype(np.float32).T
    return out


# ---------------------------------------------------------------------------
# Bass kernel
# ---------------------------------------------------------------------------

def build_bass_kernel():
    import concourse.bass as bass
    import concourse.tile as tile
    from concourse import bacc, mybir

    bf16 = mybir.dt.bfloat16
    f32 = mybir.dt.float32
    Relu = mybir.ActivationFunctionType.Relu
    Ident = mybir.ActivationFunctionType.Identity
    Alu = mybir.AluOpType

    nc = bacc.Bacc("TRN2", target_bir_lowering=False, debug=False,
                   num_devices=NCORES)
    x_dram = nc.dram_tensor("xp", [BC, SLOTS * D], bf16,
                            kind="ExternalInput").ap()
    out_dram = nc.dram_tensor("out", [J * D, BC], bf16, kind="ExternalOutput").ap()
    w1_dram = nc.dram_tensor("w1p", [128, 128 * TOTAL_CHUNKS], bf16,
                             kind="ExternalInput").ap()
    w2_dram = nc.dram_tensor("w2p", [128, 128 * 21], bf16, kind="ExternalInput").ap()
    w3_dram = nc.dram_tensor("w3p", [128, 64 * 21], bf16, kind="ExternalInput").ap()
    b1_dram = nc.dram_tensor("b1p", [128, 21], f32, kind="ExternalInput").ap()
    b2_dram = nc.dram_tensor("b2p", [128, 21], f32, kind="ExternalInput").ap()
    b3_dram = nc.dram_tensor("b3p", [128, len(L3_PAIRS)], f32,
                             kind="ExternalInput").ap()

    with tile.TileContext(nc) as tc:
        with (
            tc.tile_pool(name="wpool", bufs=1) as wpool,
            tc.tile_pool(name="xtp", bufs=2) as xtp,
            tc.tile_pool(name="actp", bufs=2) as actp,
            tc.tile_pool(name="h2p", bufs=2) as h2p,
            tc.tile_pool(name="stgp", bufs=2) as stgp,
            tc.tile_pool(name="ps1", bufs=2, space="PSUM") as ps1,
            tc.tile_pool(name="ps2", bufs=1, space="PSUM") as ps2,
            tc.tile_pool(name="ps3", bufs=2, space="PSUM") as ps3,
        ):
            w1s = wpool.tile([128, 128 * TOTAL_CHUNKS], bf16, name="w1s")
            w2s = wpool.tile([128, 128 * 21], bf16, name="w2s")
            w3s = wpool.tile([128, 64 * 21], bf16, name="w3s")
            b1s = wpool.tile([128, 21], f32, name="b1s")
            b2s = wpool.tile([128, 21], f32, name="b2s")
            b3s = wpool.tile([128, len(L3_PAIRS)], f32, name="b3s")
            nc.sync.dma_start(w1s[:], w1_dram)
            nc.sync.dma_start(w2s[:], w2_dram)
            nc.sync.dma_start(w3s[:], w3_dram)
            nc.sync.dma_start(b1s[:], b1_dram)
            nc.sync.dma_start(b2s[:], b2_dram)
            nc.sync.dma_start(b3s[:], b3_dram)

            # evac engine round-robin between the two PSUM readers
            evac_state = [0]

            def evac(dst, src, bias, relu):
                evac_state[0] ^= 1
                if evac_state[0]:
                    nc.scalar.activation(dst, src, Relu if relu else Ident,
                                         bias=bias, scale=1.0)
                else:
                    if relu:
                        nc.vector.tensor_scalar(dst, src, bias, 0.0,
                                                Alu.add, Alu.max)
                    else:
                        nc.vector.tensor_scalar(dst, src, bias, None, Alu.add)

            for it in range(NTILES):
                b0 = it * TILE
                # ONE xbar transpose for all 11 pair tiles: [TILE, 22*64]
                # -> [128, 11, TILE] (3D out: dim 1 = source column group)
                xall = xtp.tile([128, NPAIRS * TILE], bf16, tag="xall",
                                name="xall")
                nc.sync.dma_start(
                    xall[:].rearrange("p (g b) -> p g b", b=TILE),
                    x_dram[b0:b0 + TILE, :],
                    transpose=True)
                xT = [xall[:, TILE * i:TILE * (i + 1)] for i in range(NPAIRS)]

                def mlp12(t):
                    chunks = CHUNK_PLAN[t]
                    psum1 = ps1.tile([128, TILE], f32, tag="psum1", name="psum1")
                    h1 = actp.tile([128, TILE], bf16, tag="h1", name="h1")
                    # per-512-half evac: each half is its own PSUM bank, so
                    # the h0 evac overlaps the h1 matmuls and ScalarE/VectorE
                    # run in parallel on different banks
                    for h in range(TILE // 512):
                        sl = slice(512 * h, 512 * (h + 1))
                        for ci, ch in enumerate(chunks):
                            col = CHUNK_COLS[(t, ci)]
                            nc.tensor.matmul(
                                psum1[:, sl],
                                w1s[:, col:col + 128],
                                xT[ch["tile"]][:, sl],
                                start=(ci == 0), stop=(ci == len(chunks) - 1))
                        evac(h1[:, sl], psum1[:, sl], b1s[:, t:t + 1],
                             relu=True)

                    psum2 = ps2.tile([128, TILE], f32, tag="psum2", name="psum2")
                    h2 = h2p.tile([128, TILE], bf16, tag=f"h2_{t % 4}",
                                  name=f"h2_{t % 4}")
                    for h in range(TILE // 512):
                        sl = slice(512 * h, 512 * (h + 1))
                        nc.tensor.matmul(
                            psum2[:, sl],
                            w2s[:, 128 * t:128 * (t + 1)],
                            h1[:, sl],
                            start=True, stop=True)
                        evac(h2[:, sl], psum2[:, sl], b2s[:, t:t + 1],
                             relu=True)
                    return h2

                # ---- fused L1/L2 then L3 per pair of targets ----
                for pi, pr in enumerate(L3_PAIRS):
                    h2t = [mlp12(t) for t in pr]
                    m = 64 * len(pr)
                    stg = stgp.tile([m, TILE], bf16, tag=f"stg{pi}",
                                    name=f"stg{pi}")
                    for h in range(TILE // 512):
                        psum3 = ps3.tile([m, 512], f32, tag="psum3", name="psum3")
                        for k, t in enumerate(pr):
                            nc.tensor.matmul(
                                psum3[64 * k:64 * (k + 1), :],
                                w3s[:, 64 * t:64 * (t + 1)],
                                h2t[k][:, 512 * h:512 * (h + 1)],
                                start=True, stop=True,
                                skip_group_check=True)
                        evac(stg[:, 512 * h:512 * (h + 1)], psum3[:],
                             b3s[0:m, pi:pi + 1], relu=False)
                    nc.gpsimd.dma_start(
                        out_dram[128 * pi:128 * pi + m, b0:b0 + TILE], stg[:])

    nc.compile()
    return nc


PACKED = None
_NC = None
LAST_RESULT = None


def prepare(inputs):
    """Build (once) the bass module and the per-core input maps."""
    global PACKED, _NC
    import sys
    if "/opt/trn_rl_repo" not in sys.path:
        sys.path.insert(0, "/opt/trn_rl_repo")
    bf16 = ml_dtypes.bfloat16
    x = np.asarray(inputs["x"], np.float32)
    # host-side prep: bf16 cast + 22-slot pair layout [B, 22*64]
    xp = np.ascontiguousarray(x[:, SEQ, :]).astype(bf16).reshape(B, SLOTS * D)
    PACKED = pack_weights(inputs)
    if _NC is None:
        _NC = build_bass_kernel()
    in_maps = []
    for core in range(NCORES):
        m = dict(PACKED)
        m["xp"] = xp[core * BC:(core + 1) * BC]
        in_maps.append(m)
    return _NC, in_maps


def kernel(**inputs):
    global LAST_RESULT
    nc, in_maps = prepare(inputs)
    from concourse.bass_utils import run_bass_kernel_spmd
    res = run_bass_kernel_spmd(nc, in_maps, core_ids=list(range(NCORES)))
    LAST_RESULT = res
    # per-core out: [21*64, BC] bf16, feature-major
    full = np.concatenate([r["out"] for r in res.results], 1)  # [1344, B]
    return np.ascontiguousarray(
        full.reshape(J, D, B).transpose(2, 0, 1)).astype(np.float32)


# revision 12
# speedup vs baseline: 1.1273x; 1.1273x over previous
"""Trainium2 Bass kernel for the 21-joint hand-graph message-passing MLP.

Math (per sample b, per target joint t with neighbor list S_t of length n):
    g   = concat(x[b, S_t[0]], ..., x[b, S_t[n-1]])          # [n*64]
    h1  = relu(g @ W1_t + b1_t)                              # [128]
    h2  = relu(h1 @ W2_t + b2_t)                             # [128]
    out[b, t] = h2 @ W3_t + b3_t                             # [64]

Strategy (pure data parallel over 8 NeuronCores, B=65536 -> 8192/core):
  - The host pre-packs x to bf16 in a 22-slot node order SEQ so that the
    11 adjacent slot pairs are exactly the pair tiles the L1 chunk plan
    wants: (0,1),(1,2),(3,4),(5,6),(7,8),(9,10),(11,12),(13,14),(15,16),
    (17,18),(19,20).  Per batch tile, ONE xbar DMA-transpose of
    [TILE, 22*64] -> [128, 11, TILE] (3D out: dim 1 = source column
    group) produces all 11 feature-major pair tiles in a single
    instruction, keeping the Sync sequencer off the critical path.
    Per-core HBM traffic: 22.6MB read + 21.5MB write.
  - L1 runs weight-stationary with K=128 chunks: a chunk is either a
    full pair (both nodes of a tile in the neighbor list) or a single
    (one node, other 64 weight rows zero).  47 chunks total.
  - L1/L2 relu+bias are fused into the PSUM->SBUF evacuation, alternated
    between ScalarE and VectorE (the only PSUM readers).
  - L3 is W3-stationary with the output FEATURE-major: two targets share
    one PSUM bank via column tiling (psum[0:64]=target a, psum[64:128]=b),
    N=512 per matmul.  b3 is added during evacuation (per-partition
    bias).  The store is a plain bf16 DMA to out[1344, BC] on the GpSimd
    SWDGE path; the host does the final [B,21,64] transpose + fp32 cast.
"""

import os
import numpy as np
import ml_dtypes

B, J, D, H1, H2 = 65536, 21, 64, 128, 128
NCORES = 8
BC = B // NCORES          # 8192 samples per core
TILE = 1024               # batch tile (2 PSUM banks wide in fp32)
NTILES = BC // TILE       # 8

FINGER_BASE = [4 * f + 1 for f in range(5)]
NEIGH = {
    6: [[0, 1, 5, 9, 13, 17]],
    5: [[0, 5, 6, 1, 9], [0, 9, 10, 5, 13], [0, 13, 14, 9, 17]],
    4: [[0, 1, 2, 5], [0, 17, 18, 13]],
    3: [r for b in FINGER_BASE for r in ([b, b + 1, b + 2], [b + 1, b + 2, b + 3])],
    2: [[b + 2, b + 3] for b in FINGER_BASE],
}
OUT = {
    6: [0],
    5: [5, 9, 13],
    4: [1, 17],
    3: [j for b in FINGER_BASE for j in (b + 1, b + 2)],
    2: [b + 3 for b in FINGER_BASE],
}
GROUPS = [6, 5, 4, 3, 2]

# target t -> (n, row index within its group, neighbor list)
TARGET = {}
for n in GROUPS:
    for row, t in enumerate(OUT[n]):
        TARGET[t] = (n, row, list(NEIGH[n][row]))

# Pair tiles (adjacent node pairs) and the host-side 22-slot layout.
PAIRS = [(0, 1), (1, 2), (3, 4), (5, 6), (7, 8), (9, 10),
         (11, 12), (13, 14), (15, 16), (17, 18), (19, 20)]
NPAIRS = len(PAIRS)
SLOTS = 2 * NPAIRS                      # 22 node slots in xbf
SEQ = [j for p in PAIRS for j in p]     # slot s holds node SEQ[s]
PAIR_IDX = {p: i for i, p in enumerate(PAIRS)}

# node -> (tile, half) placement (first occurrence wins for singles)
NODE_SLOTS = {}
for i, (a, b) in enumerate(PAIRS):
    NODE_SLOTS.setdefault(a, []).append((i, 0))
    NODE_SLOTS.setdefault(b, []).append((i, 1))


def build_chunk_plan():
    """Per target, split neighbor positions into K=128 contraction chunks.

    chunk = dict(tile, slots) with slots = (pos_or_None for half 0,
    pos_or_None for half 1); position i covers W1 rows 64*i : 64*i+64.
    """
    plan = {}
    for t in range(21):
        n, _, S = TARGET[t]

        # exhaustive max matching of disjoint in-set pairs (n <= 6)
        best = []

        def rec(pos, used, acc):
            nonlocal best
            if len(acc) > len(best):
                best = list(acc)
            for i in range(pos, n):
                if used[i]:
                    continue
                for k in range(i + 1, n):
                    if used[k]:
                        continue
                    a, b = S[i], S[k]
                    lo, hi = min(a, b), max(a, b)
                    if (lo, hi) in PAIR_IDX:
                        pi, pk = (i, k) if a == lo else (k, i)
                        used[i] = used[k] = True
                        acc.append((PAIR_IDX[(lo, hi)], pi, pk))
                        rec(i + 1, used, acc)
                        acc.pop()
                        used[i] = used[k] = False
            return

        rec(0, [False] * n, [])
        chunks = []
        used = [False] * n
        for tile_idx, pi, pk in best:
            chunks.append(dict(tile=tile_idx, slots=(pi, pk)))
            used[pi] = used[pk] = True
        for i in range(n):
            if not used[i]:
                tile_idx, half = NODE_SLOTS[S[i]][0]
                slots = (i, None) if half == 0 else (None, i)
                chunks.append(dict(tile=tile_idx, slots=slots))
        plan[t] = chunks
    return plan


CHUNK_PLAN = build_chunk_plan()
TOTAL_CHUNKS = sum(len(v) for v in CHUNK_PLAN.values())     # 47

# deterministic column layout of packed W1 chunks: order by (t, ci)
CHUNK_COLS = {}
_col = 0
for _t in range(21):
    for _ci in range(len(CHUNK_PLAN[_t])):
        CHUNK_COLS[(_t, _ci)] = _col
        _col += 128

# L3 pairs of targets sharing one PSUM bank via column tiling
L3_PAIRS = [(2 * i, 2 * i + 1) for i in range(10)] + [(20,)]


def pack_weights(inputs):
    """Host-side prep: permute/pack all weights into a handful of flat arrays."""
    bf16 = ml_dtypes.bfloat16
    w1p = np.zeros((128, 128 * TOTAL_CHUNKS), np.float32)
    for t in range(21):
        n, row, S = TARGET[t]
        W1 = np.asarray(inputs[f"w1_g{n}"][row], np.float32)  # [n*64, 128]
        for ci, ch in enumerate(CHUNK_PLAN[t]):
            col = CHUNK_COLS[(t, ci)]
            for half, pos in enumerate(ch["slots"]):
                if pos is not None:
                    w1p[64 * half:64 * half + 64, col:col + 128] = \
                        W1[64 * pos:64 * pos + 64]
    w2p = np.zeros((128, 128 * 21), np.float32)
    w3p = np.zeros((128, 64 * 21), np.float32)
    b1p = np.zeros((128, 21), np.float32)
    b2p = np.zeros((128, 21), np.float32)
    b3p = np.zeros((128, len(L3_PAIRS)), np.float32)
    for t in range(21):
        n, row, _ = TARGET[t]
        w2p[:, 128 * t:128 * (t + 1)] = np.asarray(inputs[f"w2_g{n}"][row])
        w3p[:, 64 * t:64 * (t + 1)] = np.asarray(inputs[f"w3_g{n}"][row])
        b1p[:, t] = np.asarray(inputs[f"b1_g{n}"][row])
        b2p[:, t] = np.asarray(inputs[f"b2_g{n}"][row])
    for pi, pr in enumerate(L3_PAIRS):
        for k, t in enumerate(pr):
            n, row, _ = TARGET[t]
            b3p[64 * k:64 * k + 64, pi] = np.asarray(inputs[f"b3_g{n}"][row])
    return dict(
        w1p=w1p.astype(bf16), w2p=w2p.astype(bf16), w3p=w3p.astype(bf16),
        b1p=b1p, b2p=b2p, b3p=b3p,
    )


def numpy_emulate(inputs, x):
    """Bit-layout-faithful numpy model of what the HW kernel computes (minus
    PSUM rounding): used to validate the plan / packing offline."""
    bf16 = ml_dtypes.bfloat16
    packed = pack_weights(inputs)
    xb = np.asarray(x, np.float32).astype(bf16)          # [Bn, 21, 64]
    Bn = x.shape[0]
    xT = {}
    for i, (a, b) in enumerate(PAIRS):
        xT[i] = np.concatenate([xb[:, a], xb[:, b]], 1).T  # [128, Bn]
    out = np.zeros((Bn, 21, 64), np.float32)
    for t in range(21):
        psum1 = np.zeros((128, Bn), np.float32)
        for ci, ch in enumerate(CHUNK_PLAN[t]):
            col = CHUNK_COLS[(t, ci)]
            lhsT = packed["w1p"][:, col:col + 128].astype(np.float32)
            rhs = xT[ch["tile"]].astype(np.float32)
            psum1 += lhsT.T @ rhs
        h1 = np.maximum(psum1 + packed["b1p"][:, t:t + 1], 0).astype(bf16)
        w2 = packed["w2p"][:, 128 * t:128 * (t + 1)].astype(np.float32)
        psum2 = w2.T @ h1.astype(np.float32)
        h2 = np.maximum(psum2 + packed["b2p"][:, t:t + 1], 0).astype(bf16)
        w3 = packed["w3p"][:, 64 * t:64 * (t + 1)].astype(np.float32)
        psum3 = w3.T @ h2.astype(np.float32)             # [64, Bn]
        pi, k = (t // 2, t % 2) if t < 20 else (10, 0)
        b3 = packed["b3p"][64 * k:64 * k + 64, pi]
        out[:, t] = (psum3 + b3[:, None]).astype(bf16).astype(np.float32).T
    return out


# ---------------------------------------------------------------------------
# Bass kernel
# ---------------------------------------------------------------------------

def build_bass_kernel():
    import concourse.bass as bass
    import concourse.tile as tile
    from concourse import bacc, mybir

    bf16 = mybir.dt.bfloat16
    f32 = mybir.dt.float32
    Relu = mybir.ActivationFunctionType.Relu
    Ident = mybir.ActivationFunctionType.Identity
    Alu = mybir.AluOpType

    nc = bacc.Bacc("TRN2", target_bir_lowering=False, debug=False,
                   num_devices=NCORES)
    x_dram = nc.dram_tensor("xp", [BC, SLOTS * D], bf16,
                            kind="ExternalInput").ap()
    out_dram = nc.dram_tensor("out", [J * D, BC], bf16, kind="ExternalOutput").ap()
    w1_dram = nc.dram_tensor("w1p", [128, 128 * TOTAL_CHUNKS], bf16,
                             kind="ExternalInput").ap()
    w2_dram = nc.dram_tensor("w2p", [128, 128 * 21], bf16, kind="ExternalInput").ap()
    w3_dram = nc.dram_tensor("w3p", [128, 64 * 21], bf16, kind="ExternalInput").ap()
    b1_dram = nc.dram_tensor("b1p", [128, 21], f32, kind="ExternalInput").ap()
    b2_dram = nc.dram_tensor("b2p", [128, 21], f32, kind="ExternalInput").ap()
    b3_dram = nc.dram_tensor("b3p", [128, len(L3_PAIRS)], f32,
                             kind="ExternalInput").ap()

    with tile.TileContext(nc) as tc:
        with (
            tc.tile_pool(name="wpool", bufs=1) as wpool,
            tc.tile_pool(name="xtp", bufs=2) as xtp,
            tc.tile_pool(name="actp", bufs=2) as actp,
            tc.tile_pool(name="h2p", bufs=2) as h2p,
            tc.tile_pool(name="stgp", bufs=2) as stgp,
            tc.tile_pool(name="ps1", bufs=2, space="PSUM") as ps1,
            tc.tile_pool(name="ps2", bufs=1, space="PSUM") as ps2,
            tc.tile_pool(name="ps3", bufs=2, space="PSUM") as ps3,
        ):
            w1s = wpool.tile([128, 128 * TOTAL_CHUNKS], bf16, name="w1s")
            w2s = wpool.tile([128, 128 * 21], bf16, name="w2s")
            w3s = wpool.tile([128, 64 * 21], bf16, name="w3s")
            b1s = wpool.tile([128, 21], f32, name="b1s")
            b2s = wpool.tile([128, 21], f32, name="b2s")
            b3s = wpool.tile([128, len(L3_PAIRS)], f32, name="b3s")
            nc.sync.dma_start(w1s[:], w1_dram)
            nc.sync.dma_start(w2s[:], w2_dram)
            nc.sync.dma_start(w3s[:], w3_dram)
            nc.sync.dma_start(b1s[:], b1_dram)
            nc.sync.dma_start(b2s[:], b2_dram)
            nc.sync.dma_start(b3s[:], b3_dram)

            # evac engine round-robin between the two PSUM readers
            evac_state = [0]

            def evac(dst, src, bias, relu):
                evac_state[0] ^= 1
                if evac_state[0]:
                    nc.scalar.activation(dst, src, Relu if relu else Ident,
                                         bias=bias, scale=1.0)
                else:
                    if relu:
                        nc.vector.tensor_scalar(dst, src, bias, 0.0,
                                                Alu.add, Alu.max)
                    else:
                        nc.vector.tensor_scalar(dst, src, bias, None, Alu.add)

            for it in range(NTILES):
                b0 = it * TILE
                # ONE xbar transpose for all 11 pair tiles: [TILE, 22*64]
                # -> [128, 11, TILE] (3D out: dim 1 = source column group)
                xall = xtp.tile([128, NPAIRS * TILE], bf16, tag="xall",
                                name="xall")
                nc.sync.dma_start(
                    xall[:].rearrange("p (g b) -> p g b", b=TILE),
                    x_dram[b0:b0 + TILE, :],
                    transpose=True)
                xT = [xall[:, TILE * i:TILE * (i + 1)] for i in range(NPAIRS)]

                def mlp12(t):
                    chunks = CHUNK_PLAN[t]
                    psum1 = ps1.tile([128, TILE], f32, tag="psum1", name="psum1")
                    for h in range(TILE // 512):
                        for ci, ch in enumerate(chunks):
                            col = CHUNK_COLS[(t, ci)]
                            nc.tensor.matmul(
                                psum1[:, 512 * h:512 * (h + 1)],
                                w1s[:, col:col + 128],
                                xT[ch["tile"]][:, 512 * h:512 * (h + 1)],
                                start=(ci == 0), stop=(ci == len(chunks) - 1))
                    h1 = actp.tile([128, TILE], bf16, tag="h1", name="h1")
                    evac(h1[:], psum1[:], b1s[:, t:t + 1], relu=True)

                    psum2 = ps2.tile([128, TILE], f32, tag="psum2", name="psum2")
                    for h in range(TILE // 512):
                        nc.tensor.matmul(
                            psum2[:, 512 * h:512 * (h + 1)],
                            w2s[:, 128 * t:128 * (t + 1)],
                            h1[:, 512 * h:512 * (h + 1)],
                            start=True, stop=True)
                    h2 = h2p.tile([128, TILE], bf16, tag=f"h2_{t % 4}",
                                  name=f"h2_{t % 4}")
                    evac(h2[:], psum2[:], b2s[:, t:t + 1], relu=True)
                    return h2

                # ---- fused L1/L2 then L3 per pair of targets ----
                for pi, pr in enumerate(L3_PAIRS):
                    h2t = [mlp12(t) for t in pr]
                    m = 64 * len(pr)
                    stg = stgp.tile([m, TILE], bf16, tag=f"stg{pi}",
                                    name=f"stg{pi}")
                    for h in range(TILE // 512):
                        psum3 = ps3.tile([m, 512], f32, tag="psum3", name="psum3")
                        for k, t in enumerate(pr):
                            nc.tensor.matmul(
                                psum3[64 * k:64 * (k + 1), :],
                                w3s[:, 64 * t:64 * (t + 1)],
                                h2t[k][:, 512 * h:512 * (h + 1)],
                                start=True, stop=True,
                                skip_group_check=True)
                        evac(stg[:, 512 * h:512 * (h + 1)], psum3[:],
                             b3s[0:m, pi:pi + 1], relu=False)
                    nc.gpsimd.dma_start(
                        out_dram[128 * pi:128 * pi + m, b0:b0 + TILE], stg[:])

    nc.compile()
    return nc


PACKED = None
_NC = None
LAST_RESULT = None


def prepare(inputs):
    """Build (once) the bass module and the per-core input maps."""
    global PACKED, _NC
    import sys
    if "/opt/trn_rl_repo" not in sys.path:
        sys.path.insert(0, "/opt/trn_rl_repo")
    bf16 = ml_dtypes.bfloat16
    x = np.asarray(inputs["x"], np.float32)
    # host-side prep: bf16 cast + 22-slot pair layout [B, 22*64]
    xp = np.ascontiguousarray(x[:, SEQ, :]).astype(bf16).reshape(B, SLOTS * D)
    PACKED = pack_weights(inputs)
    if _NC is None:
        _NC = build_bass_kernel()
    in_maps = []
    for core in range(NCORES):
        m = dict(PACKED)
        m["xp"] = xp[core * BC:(core + 1) * BC]
        in_maps.append(m)
    return _NC, in_maps


def kernel(**inputs):
    global LAST_RESULT
    nc, in_maps = prepare(inputs)
    from concourse.bass_utils import run_bass_kernel_spmd
    res = run_bass_kernel_spmd(nc, in_maps, core_ids=list(range(NCORES)))
    LAST_RESULT = res
    # per-core out: [21*64, BC] bf16, feature-major
    full = np.concatenate([r["out"] for r in res.results], 1)  # [1344, B]
    return np.ascontiguousarray(
        full.reshape(J, D, B).transpose(2, 0, 1)).astype(np.float32)


# revision 14
# speedup vs baseline: 1.4182x; 1.2580x over previous
"""Trainium2 Bass kernel for the 21-joint hand-graph message-passing MLP.

Math (per sample b, per target joint t with neighbor list S_t of length n):
    g   = concat(x[b, S_t[0]], ..., x[b, S_t[n-1]])          # [n*64]
    h1  = relu(g @ W1_t + b1_t)                              # [128]
    h2  = relu(h1 @ W2_t + b2_t)                             # [128]
    out[b, t] = h2 @ W3_t + b3_t                             # [64]

Strategy (pure data parallel over 8 NeuronCores, B=65536 -> 8192/core):
  - The host pre-packs x to bf16 in a 22-slot node order SEQ so that the
    11 adjacent slot pairs are exactly the pair tiles the L1 chunk plan
    wants: (0,1),(1,2),(3,4),(5,6),(7,8),(9,10),(11,12),(13,14),(15,16),
    (17,18),(19,20).  Per batch tile, ONE xbar DMA-transpose of
    [TILE, 22*64] -> [128, 11, TILE] (3D out: dim 1 = source column
    group) produces all 11 feature-major pair tiles in a single
    instruction, keeping the Sync sequencer off the critical path.
    Per-core HBM traffic: 22.6MB read + 21.5MB write.
  - L1 runs weight-stationary with K=128 chunks: a chunk is either a
    full pair (both nodes of a tile in the neighbor list) or a single
    (one node, other 64 weight rows zero).  47 chunks total.
  - L1/L2 relu+bias are fused into the PSUM->SBUF evacuation, alternated
    between ScalarE and VectorE (the only PSUM readers).
  - L3 is W3-stationary with the output FEATURE-major: two targets share
    one PSUM bank via column tiling (psum[0:64]=target a, psum[64:128]=b),
    N=512 per matmul.  b3 is added during evacuation (per-partition
    bias).  The store is a plain bf16 DMA to out[1344, BC] on the GpSimd
    SWDGE path; the host does the final [B,21,64] transpose + fp32 cast.
"""

import os
import numpy as np
import ml_dtypes

B, J, D, H1, H2 = 65536, 21, 64, 128, 128
NCORES = 8
BC = B // NCORES          # 8192 samples per core
TILE = 1024               # batch tile (2 PSUM banks wide in fp32)
NTILES = BC // TILE       # 8

FINGER_BASE = [4 * f + 1 for f in range(5)]
NEIGH = {
    6: [[0, 1, 5, 9, 13, 17]],
    5: [[0, 5, 6, 1, 9], [0, 9, 10, 5, 13], [0, 13, 14, 9, 17]],
    4: [[0, 1, 2, 5], [0, 17, 18, 13]],
    3: [r for b in FINGER_BASE for r in ([b, b + 1, b + 2], [b + 1, b + 2, b + 3])],
    2: [[b + 2, b + 3] for b in FINGER_BASE],
}
OUT = {
    6: [0],
    5: [5, 9, 13],
    4: [1, 17],
    3: [j for b in FINGER_BASE for j in (b + 1, b + 2)],
    2: [b + 3 for b in FINGER_BASE],
}
GROUPS = [6, 5, 4, 3, 2]

# target t -> (n, row index within its group, neighbor list)
TARGET = {}
for n in GROUPS:
    for row, t in enumerate(OUT[n]):
        TARGET[t] = (n, row, list(NEIGH[n][row]))

# Pair tiles (adjacent node pairs) and the host-side 22-slot layout.
PAIRS = [(0, 1), (1, 2), (3, 4), (5, 6), (7, 8), (9, 10),
         (11, 12), (13, 14), (15, 16), (17, 18), (19, 20)]
NPAIRS = len(PAIRS)
SLOTS = 2 * NPAIRS                      # 22 node slots in xbf
SEQ = [j for p in PAIRS for j in p]     # slot s holds node SEQ[s]
PAIR_IDX = {p: i for i, p in enumerate(PAIRS)}

# node -> (tile, half) placement (first occurrence wins for singles)
NODE_SLOTS = {}
for i, (a, b) in enumerate(PAIRS):
    NODE_SLOTS.setdefault(a, []).append((i, 0))
    NODE_SLOTS.setdefault(b, []).append((i, 1))


def build_chunk_plan():
    """Per target, split neighbor positions into K=128 contraction chunks.

    chunk = dict(tile, slots) with slots = (pos_or_None for half 0,
    pos_or_None for half 1); position i covers W1 rows 64*i : 64*i+64.
    """
    plan = {}
    for t in range(21):
        n, _, S = TARGET[t]

        # exhaustive max matching of disjoint in-set pairs (n <= 6)
        best = []

        def rec(pos, used, acc):
            nonlocal best
            if len(acc) > len(best):
                best = list(acc)
            for i in range(pos, n):
                if used[i]:
                    continue
                for k in range(i + 1, n):
                    if used[k]:
                        continue
                    a, b = S[i], S[k]
                    lo, hi = min(a, b), max(a, b)
                    if (lo, hi) in PAIR_IDX:
                        pi, pk = (i, k) if a == lo else (k, i)
                        used[i] = used[k] = True
                        acc.append((PAIR_IDX[(lo, hi)], pi, pk))
                        rec(i + 1, used, acc)
                        acc.pop()
                        used[i] = used[k] = False
            return

        rec(0, [False] * n, [])
        chunks = []
        used = [False] * n
        for tile_idx, pi, pk in best:
            chunks.append(dict(tile=tile_idx, slots=(pi, pk)))
            used[pi] = used[pk] = True
        for i in range(n):
            if not used[i]:
                tile_idx, half = NODE_SLOTS[S[i]][0]
                slots = (i, None) if half == 0 else (None, i)
                chunks.append(dict(tile=tile_idx, slots=slots))
        plan[t] = chunks
    return plan


CHUNK_PLAN = build_chunk_plan()
TOTAL_CHUNKS = sum(len(v) for v in CHUNK_PLAN.values())     # 47

# deterministic column layout of packed W1 chunks: order by (t, ci)
CHUNK_COLS = {}
_col = 0
for _t in range(21):
    for _ci in range(len(CHUNK_PLAN[_t])):
        CHUNK_COLS[(_t, _ci)] = _col
        _col += 128

# L3 pairs of targets sharing one PSUM bank via column tiling
L3_PAIRS = [(2 * i, 2 * i + 1) for i in range(10)] + [(20,)]


def pack_weights(inputs):
    """Host-side prep: permute/pack all weights into a handful of flat arrays."""
    bf16 = ml_dtypes.bfloat16
    w1p = np.zeros((128, 128 * TOTAL_CHUNKS), np.float32)
    for t in range(21):
        n, row, S = TARGET[t]
        W1 = np.asarray(inputs[f"w1_g{n}"][row], np.float32)  # [n*64, 128]
        for ci, ch in enumerate(CHUNK_PLAN[t]):
            col = CHUNK_COLS[(t, ci)]
            for half, pos in enumerate(ch["slots"]):
                if pos is not None:
                    w1p[64 * half:64 * half + 64, col:col + 128] = \
                        W1[64 * pos:64 * pos + 64]
    w2p = np.zeros((128, 128 * 21), np.float32)
    w3p = np.zeros((128, 64 * 21), np.float32)
    b1p = np.zeros((128, 21), np.float32)
    b2p = np.zeros((128, 21), np.float32)
    b3p = np.zeros((128, len(L3_PAIRS)), np.float32)
    for t in range(21):
        n, row, _ = TARGET[t]
        w2p[:, 128 * t:128 * (t + 1)] = np.asarray(inputs[f"w2_g{n}"][row])
        w3p[:, 64 * t:64 * (t + 1)] = np.asarray(inputs[f"w3_g{n}"][row])
        b1p[:, t] = np.asarray(inputs[f"b1_g{n}"][row])
        b2p[:, t] = np.asarray(inputs[f"b2_g{n}"][row])
    for pi, pr in enumerate(L3_PAIRS):
        for k, t in enumerate(pr):
            n, row, _ = TARGET[t]
            b3p[64 * k:64 * k + 64, pi] = np.asarray(inputs[f"b3_g{n}"][row])
    return dict(
        w1p=w1p.astype(bf16), w2p=w2p.astype(bf16), w3p=w3p.astype(bf16),
        b1p=b1p, b2p=b2p, b3p=b3p,
    )


def numpy_emulate(inputs, x):
    """Bit-layout-faithful numpy model of what the HW kernel computes (minus
    PSUM rounding): used to validate the plan / packing offline."""
    bf16 = ml_dtypes.bfloat16
    packed = pack_weights(inputs)
    xb = np.asarray(x, np.float32).astype(bf16)          # [Bn, 21, 64]
    Bn = x.shape[0]
    xT = {}
    for i, (a, b) in enumerate(PAIRS):
        xT[i] = np.concatenate([xb[:, a], xb[:, b]], 1).T  # [128, Bn]
    out = np.zeros((Bn, 21, 64), np.float32)
    for t in range(21):
        psum1 = np.zeros((128, Bn), np.float32)
        for ci, ch in enumerate(CHUNK_PLAN[t]):
            col = CHUNK_COLS[(t, ci)]
            lhsT = packed["w1p"][:, col:col + 128].astype(np.float32)
            rhs = xT[ch["tile"]].astype(np.float32)
            psum1 += lhsT.T @ rhs
        h1 = np.maximum(psum1 + packed["b1p"][:, t:t + 1], 0).astype(bf16)
        w2 = packed["w2p"][:, 128 * t:128 * (t + 1)].astype(np.float32)
        psum2 = w2.T @ h1.astype(np.float32)
        h2 = np.maximum(psum2 + packed["b2p"][:, t:t + 1], 0).astype(bf16)
        w3 = packed["w3p"][:, 64 * t:64 * (t + 1)].astype(np.float32)
        psum3 = w3.T @ h2.astype(np.float32)             # [64, Bn]
        pi, k = (t // 2, t % 2) if t < 20 else (10, 0)
        b3 = packed["b3p"][64 * k:64 * k + 64, pi]
        out[:, t] = (psum3 + b3[:, None]).astype(bf16).astype(np.float32).T
    return out


# ---------------------------------------------------------------------------
# Bass kernel
# ---------------------------------------------------------------------------

def build_bass_kernel():
    import concourse.bass as bass
    import concourse.tile as tile
    from concourse import bacc, mybir

    bf16 = mybir.dt.bfloat16
    f32 = mybir.dt.float32
    Relu = mybir.ActivationFunctionType.Relu
    Ident = mybir.ActivationFunctionType.Identity
    Alu = mybir.AluOpType

    nc = bacc.Bacc("TRN2", target_bir_lowering=False, debug=False,
                   num_devices=NCORES)
    x_dram = nc.dram_tensor("xp", [BC, SLOTS * D], bf16,
                            kind="ExternalInput").ap()
    out_dram = nc.dram_tensor("out", [J * D, BC], bf16, kind="ExternalOutput").ap()
    w1_dram = nc.dram_tensor("w1p", [128, 128 * TOTAL_CHUNKS], bf16,
                             kind="ExternalInput").ap()
    w2_dram = nc.dram_tensor("w2p", [128, 128 * 21], bf16, kind="ExternalInput").ap()
    w3_dram = nc.dram_tensor("w3p", [128, 64 * 21], bf16, kind="ExternalInput").ap()
    b1_dram = nc.dram_tensor("b1p", [128, 21], f32, kind="ExternalInput").ap()
    b2_dram = nc.dram_tensor("b2p", [128, 21], f32, kind="ExternalInput").ap()
    b3_dram = nc.dram_tensor("b3p", [128, len(L3_PAIRS)], f32,
                             kind="ExternalInput").ap()

    with tile.TileContext(nc) as tc:
        with (
            tc.tile_pool(name="wpool", bufs=1) as wpool,
            tc.tile_pool(name="xtp", bufs=2) as xtp,
            tc.tile_pool(name="actp", bufs=3) as actp,
            tc.tile_pool(name="h2p", bufs=2) as h2p,
            tc.tile_pool(name="stgp", bufs=2) as stgp,
            tc.tile_pool(name="ps1", bufs=2, space="PSUM") as ps1,
            tc.tile_pool(name="ps2", bufs=1, space="PSUM") as ps2,
            tc.tile_pool(name="ps3", bufs=2, space="PSUM") as ps3,
        ):
            w1s = wpool.tile([128, 128 * TOTAL_CHUNKS], bf16, name="w1s")
            w2s = wpool.tile([128, 128 * 21], bf16, name="w2s")
            w3s = wpool.tile([128, 64 * 21], bf16, name="w3s")
            b1s = wpool.tile([128, 21], f32, name="b1s")
            b2s = wpool.tile([128, 21], f32, name="b2s")
            b3s = wpool.tile([128, len(L3_PAIRS)], f32, name="b3s")
            nc.sync.dma_start(w1s[:], w1_dram)
            nc.sync.dma_start(w2s[:], w2_dram)
            nc.sync.dma_start(w3s[:], w3_dram)
            nc.sync.dma_start(b1s[:], b1_dram)
            nc.sync.dma_start(b2s[:], b2_dram)
            nc.sync.dma_start(b3s[:], b3_dram)

            # evac engine round-robin between the two PSUM readers
            evac_state = [0]

            def evac(dst, src, bias, relu):
                evac_state[0] ^= 1
                if evac_state[0]:
                    nc.scalar.activation(dst, src, Relu if relu else Ident,
                                         bias=bias, scale=1.0)
                else:
                    if relu:
                        nc.vector.tensor_scalar(dst, src, bias, 0.0,
                                                Alu.add, Alu.max)
                    else:
                        nc.vector.tensor_scalar(dst, src, bias, None, Alu.add)

            # ---- software-pipelined emission across all (iter, target) ----
            # PE queue is strict FIFO, so in naive order every L2 stalls the
            # PE behind its own L1's evac.  Emit L1(g), L2(g-1), L3(pair
            # ready at g-3) instead: independent L1 chunks fill each evac
            # latency window.  Flattened across batch tiles so the pipeline
            # never drains until the very end.
            xalls = {}

            def transpose_iter(it):
                xall = xtp.tile([128, NPAIRS * TILE], bf16, tag="xall",
                                name="xall")
                nc.sync.dma_start(
                    xall[:].rearrange("p (g b) -> p g b", b=TILE),
                    x_dram[it * TILE:(it + 1) * TILE, :],
                    transpose=True)
                xalls[it] = xall

            def l1(it, t):
                xall = xalls[it]
                chunks = CHUNK_PLAN[t]
                psum1 = ps1.tile([128, TILE], f32, tag="psum1", name="psum1")
                for h in range(TILE // 512):
                    for ci, ch in enumerate(chunks):
                        col = CHUNK_COLS[(t, ci)]
                        off = TILE * ch["tile"] + 512 * h
                        nc.tensor.matmul(
                            psum1[:, 512 * h:512 * (h + 1)],
                            w1s[:, col:col + 128],
                            xall[:, off:off + 512],
                            start=(ci == 0), stop=(ci == len(chunks) - 1))
                h1 = actp.tile([128, TILE], bf16, tag="h1", name="h1")
                evac(h1[:], psum1[:], b1s[:, t:t + 1], relu=True)
                return h1

            def l2(t, h1):
                psum2 = ps2.tile([128, TILE], f32, tag="psum2", name="psum2")
                for h in range(TILE // 512):
                    nc.tensor.matmul(
                        psum2[:, 512 * h:512 * (h + 1)],
                        w2s[:, 128 * t:128 * (t + 1)],
                        h1[:, 512 * h:512 * (h + 1)],
                        start=True, stop=True)
                h2 = h2p.tile([128, TILE], bf16, tag=f"h2_{t % 4}",
                              name=f"h2_{t % 4}")
                evac(h2[:], psum2[:], b2s[:, t:t + 1], relu=True)
                return h2

            def l3(it, pi, h2t):
                pr = L3_PAIRS[pi]
                b0 = it * TILE
                m = 64 * len(pr)
                stg = stgp.tile([m, TILE], bf16, tag=f"stg{pi}",
                                name=f"stg{pi}")
                for h in range(TILE // 512):
                    psum3 = ps3.tile([m, 512], f32, tag="psum3", name="psum3")
                    for k, t in enumerate(pr):
                        nc.tensor.matmul(
                            psum3[64 * k:64 * (k + 1), :],
                            w3s[:, 64 * t:64 * (t + 1)],
                            h2t[k][:, 512 * h:512 * (h + 1)],
                            start=True, stop=True,
                            skip_group_check=True)
                    evac(stg[:, 512 * h:512 * (h + 1)], psum3[:],
                         b3s[0:m, pi:pi + 1], relu=False)
                nc.gpsimd.dma_start(
                    out_dram[128 * pi:128 * pi + m, b0:b0 + TILE], stg[:])

            sched = [(it, t) for it in range(NTILES) for t in range(21)]
            transpose_iter(0)
            prev = None          # (t, h1) awaiting L2
            h2s = {}             # (it, t) -> h2 awaiting L3
            l3q = []             # (ready_g, it, pi)
            for g, (it, t) in enumerate(sched):
                if t == 10 and it + 1 < NTILES:
                    transpose_iter(it + 1)
                h1 = l1(it, t)
                if prev is not None:
                    pit, pt, ph1 = prev
                    h2s[(pit, pt)] = l2(pt, ph1)
                    pi = pt // 2 if pt < 20 else 10
                    if pt == 2 * pi + 1 or pt == 20:
                        l3q.append((g + 2, pit, pi))
                prev = (it, t, h1)
                while l3q and l3q[0][0] <= g:
                    _, lit, lpi = l3q.pop(0)
                    pr = L3_PAIRS[lpi]
                    l3(lit, lpi, [h2s.pop((lit, tt)) for tt in pr])
            # drain
            pit, pt, ph1 = prev
            h2s[(pit, pt)] = l2(pt, ph1)
            l3q.append((0, pit, 10))
            for _, lit, lpi in l3q:
                pr = L3_PAIRS[lpi]
                l3(lit, lpi, [h2s.pop((lit, tt)) for tt in pr])

    nc.compile()
    return nc


PACKED = None
_NC = None
LAST_RESULT = None


def prepare(inputs):
    """Build (once) the bass module and the per-core input maps."""
    global PACKED, _NC
    import sys
    if "/opt/trn_rl_repo" not in sys.path:
        sys.path.insert(0, "/opt/trn_rl_repo")
    bf16 = ml_dtypes.bfloat16
    x = np.asarray(inputs["x"], np.float32)
    # host-side prep: bf16 cast + 22-slot pair layout [B, 22*64]
    xp = np.ascontiguousarray(x[:, SEQ, :]).astype(bf16).reshape(B, SLOTS * D)
    PACKED = pack_weights(inputs)
    if _NC is None:
        _NC = build_bass_kernel()
    in_maps = []
    for core in range(NCORES):
        m = dict(PACKED)
        m["xp"] = xp[core * BC:(core + 1) * BC]
        in_maps.append(m)
    return _NC, in_maps


def kernel(**inputs):
    global LAST_RESULT
    nc, in_maps = prepare(inputs)
    from concourse.bass_utils import run_bass_kernel_spmd
    res = run_bass_kernel_spmd(nc, in_maps, core_ids=list(range(NCORES)))
    LAST_RESULT = res
    # per-core out: [21*64, BC] bf16, feature-major
    full = np.concatenate([r["out"] for r in res.results], 1)  # [1344, B]
    return np.ascontiguousarray(
        full.reshape(J, D, B).transpose(2, 0, 1)).astype(np.float32)
